# revision 31
# baseline (speedup 1.0000x reference)
"""Distributed causal multi-head attention kernel for one TRN2 chip (8 NeuronCores).

Problem shapes (hardcoded): x [2, 2048, 1024], 16 heads x 64 head-dim, f32 I/O.

Sharding strategy:
  - Heads sharded 2-per-core: each core computes Q/K/V projections and causal
    attention for its 2 heads over the full sequence (perfectly balanced).
  - Scores are computed TRANSPOSED (S^T [tk, tq]) so softmax needs no
    cross-partition reduction: P' = exp(S^T/8) elementwise (no max-subtract;
    values are small enough for f32/bf16), rowsums come from a ones-column
    appended to V in the P'V matmul (lhsT M=65), normalization multiplies by
    the DMA-partition-broadcast reciprocal rowsum.
  - Per-batch AllToAll converts head-sharding -> sequence-sharding of z^T:
    each core sends 8 x [128, 256] column slices and receives exactly its
    256-row slice of the gathered z^T (1MB/core/batch round trip instead of
    the 16MB an AllGather would move), statically addressed.  Both
    collectives are emitted strictly after the attention work that feeds
    them so no engine queue head-of-line-blocks on a collective (which
    cross-core-deadlocks the mesh for ~100us).
  - x is staged host-side as per-(batch, qtile) tiles holding all 8
    contraction chunks, DMA'd in per-chunk (first tile) / per-pair slices so
    the first projection matmul starts ~4us in and never outruns HBM.
  - Output is returned in bf16 (host upcasts); all biases are applied
    exactly on device.
"""

import sys

import numpy as np
import ml_dtypes

sys.path.insert(0, "/opt/trn_rl_repo")

B, T, E, N, H = 2, 2048, 1024, 16, 64
NCORES = 8
HPC = N // NCORES          # 2 heads per core
HL = HPC * H               # 128: local head width
BT = B * T                 # 4096
ROWS = BT // NCORES        # 512: output rows per core
HROWS = ROWS // B          # 256: output rows per core per batch
EC = E // 128              # 8 chunks of the embedding (contraction) dim
GC = (N * H) // 128        # 8 chunks of the flattened head dim
TQ = 512                   # query tile (free dim of S^T / Z matmuls)
NQ = T // TQ               # 4 query tiles per batch
NKC = T // 128             # 16 key chunks per batch

BF16 = ml_dtypes.bfloat16

_CACHE = {}


def _build():
    import concourse.mybir as mybir
    from concourse import bacc
    from concourse.tile import TileContext, add_dep_helper
    from concourse.masks import make_identity

    f32 = mybir.dt.float32
    bf16 = mybir.dt.bfloat16

    nc = bacc.Bacc("TRN2", num_devices=NCORES)

    # x staged as [B, NQ, 128, EC, 512]: per (batch, qtile) all 8 chunks
    xq_d = nc.dram_tensor("xq", [B, NQ, 128, EC, TQ], bf16, kind="ExternalInput")
    # projection weights staged chunk-major along the free dim: one SBUF
    # tile per tensor, two DMAs each
    wq_d = nc.dram_tensor("wq", [128, EC, HL], bf16, kind="ExternalInput")
    wk_d = nc.dram_tensor("wk", [128, EC, HL], bf16, kind="ExternalInput")
    wv_d = nc.dram_tensor("wv", [128, EC, HL], bf16, kind="ExternalInput")
    wo_d = nc.dram_tensor("wo", [GC, 128, E], bf16, kind="ExternalInput")
    bq_d = nc.dram_tensor("bq", [HL, 1], f32, kind="ExternalInput")
    bk_d = nc.dram_tensor("bk", [HL, 1], f32, kind="ExternalInput")
    bv_d = nc.dram_tensor("bv", [HL, 1], f32, kind="ExternalInput")
    bo_d = nc.dram_tensor("bo", [1, E], bf16, kind="ExternalInput")
    cm_d = nc.dram_tensor("cmask", [128, 128], bf16, kind="ExternalInput")
    out_d = nc.dram_tensor("out", [ROWS, E], bf16, kind="ExternalOutput")
    a2a_in = [
        nc.dram_tensor(f"a2a_in{b}", [NCORES, HL, HROWS], bf16, kind="Internal")
        for b in range(B)
    ]
    a2a_out = [
        nc.dram_tensor(f"a2a_out{b}", [NCORES, HL, HROWS], bf16, kind="Internal")
        for b in range(B)
    ]

    with TileContext(nc) as tc:
        with (
            tc.tile_pool(name="singles", bufs=1) as singles,
            tc.tile_pool(name="ptiles", bufs=6) as ptiles,
            tc.tile_pool(name="ztiles", bufs=6) as ztiles,
            tc.tile_pool(name="rtiles", bufs=4) as rtiles,
            tc.tile_pool(name="otiles", bufs=4) as otiles,
            tc.tile_pool(name="dscratch", bufs=8, space="DRAM") as dscratch,
            tc.tile_pool(name="psum", bufs=4, space="PSUM") as psum,
            tc.tile_pool(name="psum2", bufs=2, space="PSUM") as psum2,
        ):
            # ---- input DMAs, ordered for earliest possible first matmul ----
            # (the sync sequencer serializes dma_starts at ~600ns each, so
            # both the count and the order matter)
            # tiny warm-up collective: absorbs the first-collective mesh
            # channel setup (~15us) during the projection phase so the real
            # exchanges later see only ~3-5us of barrier latency
            warm_in = nc.dram_tensor(
                "warm_in", [NCORES, 1, 16], bf16, kind="Internal"
            )
            warm_out = nc.dram_tensor(
                "warm_out", [NCORES, 1, 16], bf16, kind="Internal"
            )
            nc.gpsimd.collective_compute(
                "AllToAll",
                mybir.AluOpType.bypass,
                replica_groups=[list(range(NCORES))],
                ins=[warm_in[:]],
                outs=[warm_out[:]],
            )

            bq = singles.tile([HL, 1], f32)
            bk = singles.tile([HL, 1], f32)
            bv = singles.tile([HL, 1], f32)
            cm = singles.tile([128, 128], bf16)

            wqs = singles.tile([128, EC, HL], bf16, name="wqs")
            wks = singles.tile([128, EC, HL], bf16, name="wks")
            wvs = singles.tile([128, EC, HL], bf16, name="wvs")
            wq = [wqs[:, k, :] for k in range(EC)]
            wk = [wks[:, k, :] for k in range(EC)]
            wv = [wvs[:, k, :] for k in range(EC)]
            xq = [
                [
                    singles.tile([128, EC, TQ], bf16, name=f"xq{b}_{qt}")
                    for qt in range(NQ)
                ]
                for b in range(B)
            ]
            # first-compute chain first: wq half-a + x chunk 0, then the rest
            # of the weights interleaved with batch 0's first qtile chunks
            # (each split in two so transfers land on two DMA engines and the
            # k-th accumulation step fires as it lands)
            HE = EC // 2
            HQ = TQ // 2

            def _xk(k):
                for s in range(2):
                    nc.sync.dma_start(
                        out=xq[0][0][:, k, s * HQ : (s + 1) * HQ],
                        in_=xq_d[0, 0][:, k, s * HQ : (s + 1) * HQ],
                    )

            def _wh(w, wd, s):
                nc.sync.dma_start(
                    out=w[:, s * HE : (s + 1) * HE, :],
                    in_=wd[:, s * HE : (s + 1) * HE, :],
                )

            _wh(wqs, wq_d, 0)
            _xk(0)
            _wh(wqs, wq_d, 1)
            _wh(wks, wk_d, 0)
            _xk(1)
            _wh(wks, wk_d, 1)
            _wh(wvs, wv_d, 0)
            _xk(2)
            _wh(wvs, wv_d, 1)
            nc.sync.dma_start(out=bq, in_=bq_d[:])
            nc.sync.dma_start(out=bk, in_=bk_d[:])
            nc.sync.dma_start(out=bv, in_=bv_d[:])
            _xk(3)
            nc.sync.dma_start(out=cm, in_=cm_d[:])
            for k in range(4, EC):
                _xk(k)
            # remaining x tiles: 2-chunk slices (256KB per DMA) round-robin
            # across the DMA engines
            for b in range(B):
                for qt in range(NQ):
                    if b == 0 and qt == 0:
                        continue
                    for k in range(0, EC, 2):
                        nc.sync.dma_start(
                            out=xq[b][qt][:, k : k + 2, :],
                            in_=xq_d[b, qt][:, k : k + 2, :],
                        )
            ident = singles.tile([128, 128], bf16)
            make_identity(nc, ident)

            # O-projection weights: loaded during the attention phase's idle
            # DMA time (needed right after the first AllToAll)
            wo = singles.tile([128, GC, E], bf16)
            for g in range(GC):
                nc.sync.dma_start(out=wo[:, g, :], in_=wo_d[g])
            # output bias pre-broadcast to all partitions (adds on DVE during
            # the psum->sbuf copy; cheaper than a ones-row bias matmul)
            bob = singles.tile([128, E], bf16)
            nc.sync.dma_start(out=bob, in_=bo_d[0:1, :].partition_broadcast(128))

            # ---- Q^T / K^T / V^T projections: [128(2hxH), T] per batch ----
            qT = [singles.tile([128, T], bf16, name=f"qT{b}") for b in range(B)]
            kT = [singles.tile([128, T], bf16, name=f"kT{b}") for b in range(B)]
            vT = [singles.tile([128, T], bf16, name=f"vT{b}") for b in range(B)]
            vp = [
                singles.tile([128, NKC, HPC, H + 1], bf16, name=f"vp{b}")
                for b in range(B)
            ]
            for b in range(B):
                nc.vector.memset(vp[b][:, :, :, H : H + 1], 1.0)
            for b in range(B):
                # qtile-outer / projection-inner so attention's first query
                # tile unblocks as early as possible
                for qt in range(NQ):
                    for w, dst, bias in (
                        (wq, qT[b], bq),
                        (wk, kT[b], bk),
                        (wv, vT[b], bv),
                    ):
                        ps = psum.tile([128, TQ], f32, tag="ps", name="ps_prj")
                        for k in range(EC):
                            nc.tensor.matmul(
                                ps,
                                lhsT=w[k],
                                rhs=xq[b][qt][:, k, :],
                                start=(k == 0),
                                stop=(k == EC - 1),
                            )
                        # psum -> sbuf cast with exact per-partition bias add
                        nc.vector.tensor_scalar_add(
                            dst[:, qt * TQ : (qt + 1) * TQ], ps, bias
                        )
                    # V' = [V | ones] for this qtile's 4 key chunks:
                    # vp [128(tk), chunk, head, 65]
                    for c in range(4 * qt, 4 * qt + 4):
                        pst = psum.tile([128, 128], bf16, tag="ps", name="ps_tr")
                        nc.tensor.transpose(
                            pst,
                            in_=vT[b][:, c * 128 : (c + 1) * 128],
                            identity=ident,
                        )
                        for h in range(HPC):
                            nc.vector.tensor_copy(
                                out=vp[b][:, c, h, 0:H],
                                in_=pst[:, h * H : (h + 1) * H],
                            )

            # ---- attention per (batch, query-tile), batches interleaved ----
            # alternating (b0,q),(b1,q) gives the PE a second independent
            # chunk stream to fill bubbles whenever one qtile's exp/mask
            # chain stalls; the psum pool holds exactly two qtiles in flight
            zz_stores = [[] for _ in range(B)]
            ccs = [None] * B

            def emit_cc(b):
                cc = nc.gpsimd.collective_compute(
                    "AllToAll",
                    mybir.AluOpType.bypass,
                    replica_groups=[list(range(NCORES))],
                    ins=[a2a_in[b][:]],
                    outs=[a2a_out[b][:]],
                )
                for d in zz_stores[b]:
                    add_dep_helper(cc.ins, d.ins, reason="a2a after z stores")
                ccs[b] = cc

            for q in range(NQ):
                for b in range(B):
                    zps = [
                        psum.tile([128, TQ], f32, tag="ps", name=f"zps{h}")
                        for h in range(HPC)
                    ]
                    nkeep = 4 * q + 4  # causal: key chunks 0..4q+3
                    for c in range(nkeep):
                        # diagonal chunks (j>=0): columns < j*128 are fully
                        # masked -> clip them out of S/exp/mask/Z entirely
                        j = c - 4 * q
                        lo = j * 128 if j >= 0 else 0
                        # both heads' scores into ONE 2-bank psum tile so a
                        # single exp covers them (amortizes ACT op overhead);
                        # the S pair runs concurrently in disjoint row groups
                        sps = psum2.tile([128, 2 * TQ], f32, tag="ps2", name="sps")
                        kcols = slice(c * 128, (c + 1) * 128)
                        for h in range(HPC):
                            hp = slice(h * H, (h + 1) * H)
                            nc.tensor.matmul(
                                sps[:, h * TQ + lo : (h + 1) * TQ],
                                lhsT=kT[b][hp, kcols],
                                rhs=qT[b][hp, q * TQ + lo : (q + 1) * TQ],
                                start=True,
                                stop=True,
                                tile_position=(h * H, 0),
                            )
                        pp = ptiles.tile([128, 2 * TQ], bf16, tag="pp")
                        if j < 3:
                            nc.scalar.activation(
                                pp[:, lo : 2 * TQ],
                                sps[:, lo : 2 * TQ],
                                mybir.ActivationFunctionType.Exp,
                                scale=0.125,
                            )
                        else:  # j=3: two ops beat exp-ing the 384-col gap
                            for h in range(HPC):
                                nc.scalar.activation(
                                    pp[:, h * TQ + lo : (h + 1) * TQ],
                                    sps[:, h * TQ + lo : (h + 1) * TQ],
                                    mybir.ActivationFunctionType.Exp,
                                    scale=0.125,
                                )
                        if j >= 0:  # causal mask on the diagonal blocks
                            # (must stay on DVE: gpsimd would thrash its DSP
                            # library against partition_broadcast's)
                            for h in range(HPC):
                                nc.vector.tensor_mul(
                                    pp[:, h * TQ + lo : h * TQ + lo + 128],
                                    pp[:, h * TQ + lo : h * TQ + lo + 128],
                                    cm,
                                )
                        for h in range(HPC):
                            nc.tensor.matmul(
                                zps[h][0 : H + 1, lo:],
                                lhsT=vp[b][:, c, h, :],
                                rhs=pp[:, h * TQ + lo : (h + 1) * TQ],
                                start=(c == 0),
                                stop=(c == nkeep - 1),
                            )
                    # copy z' (and its rowsum row) out of PSUM immediately so
                    # the zps banks recycle for the next qtile's matmuls
                    # instead of being held hostage by the normalize chain
                    zft = ztiles.tile([H + 1, HPC, TQ], f32, tag="zft", name="zft")
                    for h in range(HPC):
                        nc.vector.tensor_copy(
                            out=zft[:, h, :], in_=zps[h][0 : H + 1, :]
                        )
                    # normalize: z = z' * (1/rowsum).  Reshape [1, 2*TQ] onto
                    # 128 partitions with an SBUF->SBUF DMA so the DVE
                    # reciprocal runs wide (8/partition, ~0.2us -- a [1,512]
                    # 1-partition reciprocal costs 3.3us), reshape back, then
                    # a gpsimd partition-broadcast fans the result out
                    # (SBUF->SBUF, ~1.8us vs 11.4us for a 256KB DMA broadcast)
                    rq = rtiles.tile([128, 2 * TQ // 128], f32, tag="rq")
                    nc.sync.dma_start(out=rq, in_=zft[H : H + 1, :, :])
                    rqr = rtiles.tile([128, 2 * TQ // 128], f32, tag="rqr")
                    nc.vector.reciprocal(out=rqr, in_=rq)
                    rs2 = rtiles.tile([1, 2 * TQ], f32, tag="rs2")
                    nc.sync.dma_start(out=rs2, in_=rqr)
                    rbc = rtiles.tile([H, 2 * TQ], f32, tag="rbc")
                    nc.gpsimd.partition_broadcast(rbc, rs2[0:1, :])
                    for h in range(HPC):
                        zz = ztiles.tile([H, TQ], bf16, tag="zz")
                        nc.vector.tensor_mul(
                            zz, zft[0:H, h, :], rbc[:, h * TQ : (h + 1) * TQ]
                        )
                        # scatter z^T columns by destination core (2 slots of
                        # 256 per query tile)
                        for s in range(2):
                            zz_stores[b].append(
                                nc.sync.dma_start(
                                    out=a2a_in[b][
                                        2 * q + s, h * H : (h + 1) * H, :
                                    ],
                                    in_=zz[:, s * HROWS : (s + 1) * HROWS],
                                )
                            )
                    # batch 0's AllToAll goes on the gpsimd stream right
                    # after its last qtile (before batch 1's final
                    # partition_broadcast) so it fires as soon as batch 0's
                    # stores land; batch 1's comes after everything
                    if q == NQ - 1 and b == 0:
                        emit_cc(0)
            emit_cc(1)

            # ---- output projection for this core's 256-row slice of each
            # batch (batch 0's deps resolve during batch 1's attention) ----
            for b in range(B):
                zo = [
                    singles.tile([128, HROWS], bf16, name=f"zo{b}_{g}")
                    for g in range(GC)
                ]
                for g in range(GC):
                    for s in range(2):
                        zd = nc.sync.dma_start(
                            out=zo[g][:, s * 128 : (s + 1) * 128],
                            in_=a2a_out[b][g][:, s * 128 : (s + 1) * 128],
                        )
                        add_dep_helper(
                            zd.ins, ccs[b].ins, reason="zo after a2a"
                        )
                        if b == 0:
                            # hold batch 0's zo burst until batch 1's z stores
                            # are out, so these transfers fill the second
                            # collective's barrier window instead of clogging
                            # the DMA queues under batch 1's final normalize
                            add_dep_helper(
                                zd.ins, zz_stores[1][-1].ins,
                                reason="zo0 after b1 z stores",
                            )
                for rt in range(HROWS // 128):
                    for eh in range(E // 512):
                        po = psum.tile([128, 512], f32, tag="ps")
                        for g in range(GC):
                            nc.tensor.matmul(
                                po,
                                lhsT=zo[g][:, rt * 128 : (rt + 1) * 128],
                                rhs=wo[:, g, eh * 512 : (eh + 1) * 512],
                                start=(g == 0),
                                stop=(g == GC - 1),
                            )
                        ob = otiles.tile([128, 512], bf16, tag="ob")
                        nc.vector.tensor_add(
                            ob, po, bob[:, eh * 512 : (eh + 1) * 512]
                        )
                        # split the row-tile store so the tail transfer is
                        # short and lands on four DMA engines
                        r0 = b * HROWS + rt * 128
                        for s in range(4):
                            nc.sync.dma_start(
                                out=out_d[
                                    r0 + s * 32 : r0 + (s + 1) * 32,
                                    eh * 512 : (eh + 1) * 512,
                                ],
                                in_=ob[s * 32 : (s + 1) * 32, :],
                            )

    nc.compile()
    return nc


def _prep_inputs(x, W_Q, W_K, W_V, W_O, b_Q, b_K, b_V, b_O):
    # xq [B, NQ, 128(p), EC(k), 512(j)] with element [b,q,p,k,j] =
    # x[b, q*512+j, k*128+p]
    xq = np.ascontiguousarray(
        x.reshape(B, NQ, TQ, EC, 128).transpose(0, 1, 4, 3, 2).astype(BF16)
    )
    wo = np.ascontiguousarray(
        W_O.reshape(N * H, E).reshape(GC, 128, E).astype(BF16)
    )
    bo = np.ascontiguousarray(b_O.reshape(1, E).astype(BF16))
    # causal mask for the 128-col diagonal block of a chunk: allow q >= k
    cols = np.arange(128)[None, :]
    rows = np.arange(128)[:, None]
    cmask = np.ascontiguousarray((cols >= rows).astype(BF16))

    in_maps = []
    for i in range(NCORES):
        hs = slice(HPC * i, HPC * (i + 1))
        m = {
            "xq": xq,
            "wo": wo,
            "bo": bo,
            "cmask": cmask,
            "wq": np.ascontiguousarray(
                W_Q[hs]
                .transpose(1, 0, 2)
                .reshape(EC, 128, HL)
                .transpose(1, 0, 2)
                .astype(BF16)
            ),
            "wk": np.ascontiguousarray(
                W_K[hs]
                .transpose(1, 0, 2)
                .reshape(EC, 128, HL)
                .transpose(1, 0, 2)
                .astype(BF16)
            ),
            "wv": np.ascontiguousarray(
                W_V[hs]
                .transpose(1, 0, 2)
                .reshape(EC, 128, HL)
                .transpose(1, 0, 2)
                .astype(BF16)
            ),
            "bq": np.ascontiguousarray(
                b_Q[hs].reshape(HL, 1).astype(np.float32)
            ),
            "bk": np.ascontiguousarray(
                b_K[hs].reshape(HL, 1).astype(np.float32)
            ),
            "bv": np.ascontiguousarray(
                b_V[hs].reshape(HL, 1).astype(np.float32)
            ),
        }
        in_maps.append(m)
    return in_maps


def run(inputs, trace=False):
    from concourse import bass_utils

    if "nc" not in _CACHE:
        _CACHE["nc"] = _build()
    nc = _CACHE["nc"]
    in_maps = _prep_inputs(**inputs)
    res = bass_utils.run_bass_kernel_spmd(
        nc, in_maps, core_ids=list(range(NCORES)), trace=trace
    )
    out = np.zeros((B, T, E), dtype=np.float32)
    for i, r in enumerate(res.results):
        ro = np.asarray(r["out"]).astype(np.float32)
        for b in range(B):
            out[b, i * HROWS : (i + 1) * HROWS] = ro[
                b * HROWS : (b + 1) * HROWS
            ]
    return out, res


def kernel(**inputs):
    out, _ = run(inputs, trace=False)
    return out


# revision 34
# speedup vs baseline: 1.2868x; 1.2868x over previous
"""Distributed causal multi-head attention kernel for one TRN2 chip (8 NeuronCores).

Problem shapes (hardcoded): x [2, 2048, 1024], 16 heads x 64 head-dim, f32 I/O.

Sharding strategy:
  - Heads sharded 2-per-core: each core computes Q/K/V projections and causal
    attention for its 2 heads over the full sequence (perfectly balanced).
  - Scores are computed TRANSPOSED (S^T [tk, tq]) so softmax needs no
    cross-partition reduction: P' = exp(S^T/8) elementwise (no max-subtract;
    values are small enough for f32/bf16), rowsums come from a ones-column
    appended to V in the P'V matmul (lhsT M=65), normalization multiplies by
    the DMA-partition-broadcast reciprocal rowsum.
  - Per-batch AllToAll converts head-sharding -> sequence-sharding of z^T:
    each core sends 8 x [128, 256] column slices and receives exactly its
    256-row slice of the gathered z^T (1MB/core/batch round trip instead of
    the 16MB an AllGather would move), statically addressed.  Both
    collectives are emitted strictly after the attention work that feeds
    them so no engine queue head-of-line-blocks on a collective (which
    cross-core-deadlocks the mesh for ~100us).
  - x is staged host-side as per-(batch, qtile) tiles holding all 8
    contraction chunks, DMA'd in per-chunk (first tile) / per-pair slices so
    the first projection matmul starts ~4us in and never outruns HBM.
  - Output is returned in bf16 (host upcasts); all biases are applied
    exactly on device.
"""

import sys

import numpy as np
import ml_dtypes

sys.path.insert(0, "/opt/trn_rl_repo")

B, T, E, N, H = 2, 2048, 1024, 16, 64
NCORES = 8
HPC = N // NCORES          # 2 heads per core
HL = HPC * H               # 128: local head width
BT = B * T                 # 4096
ROWS = BT // NCORES        # 512: output rows per core
HROWS = ROWS // B          # 256: output rows per core per batch
EC = E // 128              # 8 chunks of the embedding (contraction) dim
GC = (N * H) // 128        # 8 chunks of the flattened head dim
TQ = 512                   # query tile (free dim of S^T / Z matmuls)
NQ = T // TQ               # 4 query tiles per batch
NKC = T // 128             # 16 key chunks per batch

BF16 = ml_dtypes.bfloat16

_CACHE = {}


def _build():
    import concourse.mybir as mybir
    from concourse import bacc
    from concourse.tile import TileContext, add_dep_helper
    from concourse.masks import make_identity

    f32 = mybir.dt.float32
    bf16 = mybir.dt.bfloat16

    nc = bacc.Bacc("TRN2", num_devices=NCORES)

    # x staged as [B, NQ, 128, EC, 512]: per (batch, qtile) all 8 chunks
    xq_d = nc.dram_tensor("xq", [B, NQ, 128, EC, TQ], bf16, kind="ExternalInput")
    # projection weights staged chunk-major along the free dim: one SBUF
    # tile per tensor, two DMAs each
    wq_d = nc.dram_tensor("wq", [128, EC, HL], bf16, kind="ExternalInput")
    wk_d = nc.dram_tensor("wk", [128, EC, HL], bf16, kind="ExternalInput")
    wv_d = nc.dram_tensor("wv", [128, EC, HL], bf16, kind="ExternalInput")
    wo_d = nc.dram_tensor("wo", [GC, 128, E], bf16, kind="ExternalInput")
    bq_d = nc.dram_tensor("bq", [HL, 1], f32, kind="ExternalInput")
    bk_d = nc.dram_tensor("bk", [HL, 1], f32, kind="ExternalInput")
    bv_d = nc.dram_tensor("bv", [HL, 1], f32, kind="ExternalInput")
    bo_d = nc.dram_tensor("bo", [1, E], bf16, kind="ExternalInput")
    cm_d = nc.dram_tensor("cmask", [128, 128], bf16, kind="ExternalInput")
    out_d = nc.dram_tensor("out", [ROWS, E], bf16, kind="ExternalOutput")
    a2a_in = [
        nc.dram_tensor(f"a2a_in{b}", [NCORES, HL, HROWS], bf16, kind="Internal")
        for b in range(B)
    ]
    a2a_out = [
        nc.dram_tensor(f"a2a_out{b}", [NCORES, HL, HROWS], bf16, kind="Internal")
        for b in range(B)
    ]

    with TileContext(nc) as tc:
        with (
            tc.tile_pool(name="singles", bufs=1) as singles,
            tc.tile_pool(name="ptiles", bufs=6) as ptiles,
            tc.tile_pool(name="ztiles", bufs=6) as ztiles,
            tc.tile_pool(name="rtiles", bufs=4) as rtiles,
            tc.tile_pool(name="otiles", bufs=4) as otiles,
            tc.tile_pool(name="dscratch", bufs=8, space="DRAM") as dscratch,
            tc.tile_pool(name="psum", bufs=4, space="PSUM") as psum,
            tc.tile_pool(name="psum2", bufs=2, space="PSUM") as psum2,
        ):
            # ---- input DMAs, ordered for earliest possible first matmul ----
            # (the sync sequencer serializes dma_starts at ~600ns each, so
            # both the count and the order matter)
            # tiny warm-up collective: absorbs the first-collective mesh
            # channel setup (~15us) during the projection phase so the real
            # exchanges later see only ~3-5us of barrier latency
            warm_in = nc.dram_tensor(
                "warm_in", [NCORES, 1, 16], bf16, kind="Internal"
            )
            warm_out = nc.dram_tensor(
                "warm_out", [NCORES, 1, 16], bf16, kind="Internal"
            )
            nc.gpsimd.collective_compute(
                "AllToAll",
                mybir.AluOpType.bypass,
                replica_groups=[list(range(NCORES))],
                ins=[warm_in[:]],
                outs=[warm_out[:]],
            )

            bq = singles.tile([HL, 1], f32)
            bk = singles.tile([HL, 1], f32)
            bv = singles.tile([HL, 1], f32)
            cm = singles.tile([128, 128], bf16)

            wqs = singles.tile([128, EC, HL], bf16, name="wqs")
            wks = singles.tile([128, EC, HL], bf16, name="wks")
            wvs = singles.tile([128, EC, HL], bf16, name="wvs")
            wq = [wqs[:, k, :] for k in range(EC)]
            wk = [wks[:, k, :] for k in range(EC)]
            wv = [wvs[:, k, :] for k in range(EC)]
            xq = [
                [
                    singles.tile([128, EC, TQ], bf16, name=f"xq{b}_{qt}")
                    for qt in range(NQ)
                ]
                for b in range(B)
            ]
            # first-compute chain first: wq half-a + x chunk 0, then the rest
            # of the weights interleaved with batch 0's first qtile chunks
            # (each split in two so transfers land on two DMA engines and the
            # k-th accumulation step fires as it lands)
            HE = EC // 2
            HQ = TQ // 2

            def _xk(k):
                for s in range(2):
                    nc.sync.dma_start(
                        out=xq[0][0][:, k, s * HQ : (s + 1) * HQ],
                        in_=xq_d[0, 0][:, k, s * HQ : (s + 1) * HQ],
                    )

            def _wh(w, wd, s):
                nc.sync.dma_start(
                    out=w[:, s * HE : (s + 1) * HE, :],
                    in_=wd[:, s * HE : (s + 1) * HE, :],
                )

            _wh(wqs, wq_d, 0)
            _xk(0)
            _wh(wqs, wq_d, 1)
            _wh(wks, wk_d, 0)
            _xk(1)
            _wh(wks, wk_d, 1)
            _wh(wvs, wv_d, 0)
            _xk(2)
            _wh(wvs, wv_d, 1)
            nc.sync.dma_start(out=bq, in_=bq_d[:])
            nc.sync.dma_start(out=bk, in_=bk_d[:])
            nc.sync.dma_start(out=bv, in_=bv_d[:])
            _xk(3)
            nc.sync.dma_start(out=cm, in_=cm_d[:])
            for k in range(4, EC):
                _xk(k)
            # remaining x tiles: 2-chunk slices (256KB per DMA) round-robin
            # across the DMA engines
            for b in range(B):
                for qt in range(NQ):
                    if b == 0 and qt == 0:
                        continue
                    for k in range(0, EC, 2):
                        nc.sync.dma_start(
                            out=xq[b][qt][:, k : k + 2, :],
                            in_=xq_d[b, qt][:, k : k + 2, :],
                        )
            ident = singles.tile([128, 128], bf16)
            make_identity(nc, ident)

            # O-projection weights: loaded during the attention phase's idle
            # DMA time (needed right after the first AllToAll)
            wo = singles.tile([128, GC, E], bf16)
            for g in range(GC):
                nc.sync.dma_start(out=wo[:, g, :], in_=wo_d[g])
            # output bias pre-broadcast to all partitions (adds on DVE during
            # the psum->sbuf copy; cheaper than a ones-row bias matmul)
            bob = singles.tile([128, E], bf16)
            nc.sync.dma_start(out=bob, in_=bo_d[0:1, :].partition_broadcast(128))

            # ---- Q^T / K^T / V^T projections: [128(2hxH), T] per batch ----
            qT = [singles.tile([128, T], bf16, name=f"qT{b}") for b in range(B)]
            kT = [singles.tile([128, T], bf16, name=f"kT{b}") for b in range(B)]
            vT = [singles.tile([128, T], bf16, name=f"vT{b}") for b in range(B)]
            vp = [
                singles.tile([128, NKC, HPC, H + 1], bf16, name=f"vp{b}")
                for b in range(B)
            ]
            for b in range(B):
                nc.vector.memset(vp[b][:, :, :, H : H + 1], 1.0)
            for b in range(B):
                # qtile-outer / projection-inner so attention's first query
                # tile unblocks as early as possible
                for qt in range(NQ):
                    for w, dst, bias in (
                        (wq, qT[b], bq),
                        (wk, kT[b], bk),
                        (wv, vT[b], bv),
                    ):
                        ps = psum.tile([128, TQ], f32, tag="ps", name="ps_prj")
                        for k in range(EC):
                            nc.tensor.matmul(
                                ps,
                                lhsT=w[k],
                                rhs=xq[b][qt][:, k, :],
                                start=(k == 0),
                                stop=(k == EC - 1),
                            )
                        # psum -> sbuf cast with exact per-partition bias add
                        nc.vector.tensor_scalar_add(
                            dst[:, qt * TQ : (qt + 1) * TQ], ps, bias
                        )
                    # V' = [V | ones] for this qtile's 4 key chunks:
                    # vp [128(tk), chunk, head, 65]
                    for c in range(4 * qt, 4 * qt + 4):
                        pst = psum.tile([128, 128], bf16, tag="ps", name="ps_tr")
                        nc.tensor.transpose(
                            pst,
                            in_=vT[b][:, c * 128 : (c + 1) * 128],
                            identity=ident,
                        )
                        for h in range(HPC):
                            nc.vector.tensor_copy(
                                out=vp[b][:, c, h, 0:H],
                                in_=pst[:, h * H : (h + 1) * H],
                            )

            # ---- attention per (batch, query-tile) ----
            zz_stores = [[] for _ in range(B)]
            ccs = [None] * B

            def emit_cc(b):
                cc = nc.gpsimd.collective_compute(
                    "AllToAll",
                    mybir.AluOpType.bypass,
                    replica_groups=[list(range(NCORES))],
                    ins=[a2a_in[b][:]],
                    outs=[a2a_out[b][:]],
                )
                for d in zz_stores[b]:
                    add_dep_helper(cc.ins, d.ins, reason="a2a after z stores")
                ccs[b] = cc

            for b in range(B):
                for q in range(NQ):
                    zps = [
                        psum.tile([128, TQ], f32, tag="ps", name=f"zps{h}")
                        for h in range(HPC)
                    ]
                    nkeep = 4 * q + 4  # causal: key chunks 0..4q+3
                    for c in range(nkeep):
                        # diagonal chunks (j>=0): columns < j*128 are fully
                        # masked -> clip them out of S/exp/mask/Z entirely
                        j = c - 4 * q
                        lo = j * 128 if j >= 0 else 0
                        # both heads' scores into ONE 2-bank psum tile so a
                        # single exp covers them (amortizes ACT op overhead);
                        # the S pair runs concurrently in disjoint row groups
                        sps = psum2.tile([128, 2 * TQ], f32, tag="ps2", name="sps")
                        kcols = slice(c * 128, (c + 1) * 128)
                        for h in range(HPC):
                            hp = slice(h * H, (h + 1) * H)
                            nc.tensor.matmul(
                                sps[:, h * TQ + lo : (h + 1) * TQ],
                                lhsT=kT[b][hp, kcols],
                                rhs=qT[b][hp, q * TQ + lo : (q + 1) * TQ],
                                start=True,
                                stop=True,
                                tile_position=(h * H, 0),
                            )
                        pp = ptiles.tile([128, 2 * TQ], bf16, tag="pp")
                        if j < 3:
                            nc.scalar.activation(
                                pp[:, lo : 2 * TQ],
                                sps[:, lo : 2 * TQ],
                                mybir.ActivationFunctionType.Exp,
                                scale=0.125,
                            )
                        else:  # j=3: two ops beat exp-ing the 384-col gap
                            for h in range(HPC):
                                nc.scalar.activation(
                                    pp[:, h * TQ + lo : (h + 1) * TQ],
                                    sps[:, h * TQ + lo : (h + 1) * TQ],
                                    mybir.ActivationFunctionType.Exp,
                                    scale=0.125,
                                )
                        if j >= 0:  # causal mask on the diagonal blocks
                            # (must stay on DVE: gpsimd would thrash its DSP
                            # library against partition_broadcast's)
                            for h in range(HPC):
                                nc.vector.tensor_mul(
                                    pp[:, h * TQ + lo : h * TQ + lo + 128],
                                    pp[:, h * TQ + lo : h * TQ + lo + 128],
                                    cm,
                                )
                        for h in range(HPC):
                            nc.tensor.matmul(
                                zps[h][0 : H + 1, lo:],
                                lhsT=vp[b][:, c, h, :],
                                rhs=pp[:, h * TQ + lo : (h + 1) * TQ],
                                start=(c == 0),
                                stop=(c == nkeep - 1),
                            )
                    # copy z' (and its rowsum row) out of PSUM immediately so
                    # the zps banks recycle for the next qtile's matmuls
                    # instead of being held hostage by the normalize chain
                    zft = ztiles.tile([H + 1, HPC, TQ], f32, tag="zft", name="zft")
                    for h in range(HPC):
                        nc.vector.tensor_copy(
                            out=zft[:, h, :], in_=zps[h][0 : H + 1, :]
                        )
                    # normalize: z = z' * (1/rowsum).  Reshape [1, 2*TQ] onto
                    # 128 partitions with an SBUF->SBUF DMA so the DVE
                    # reciprocal runs wide (8/partition, ~0.2us -- a [1,512]
                    # 1-partition reciprocal costs 3.3us), reshape back, then
                    # a gpsimd partition-broadcast fans the result out
                    # (SBUF->SBUF, ~1.8us vs 11.4us for a 256KB DMA broadcast)
                    rq = rtiles.tile([128, 2 * TQ // 128], f32, tag="rq")
                    nc.sync.dma_start(out=rq, in_=zft[H : H + 1, :, :])
                    rqr = rtiles.tile([128, 2 * TQ // 128], f32, tag="rqr")
                    nc.vector.reciprocal(out=rqr, in_=rq)
                    rs2 = rtiles.tile([1, 2 * TQ], f32, tag="rs2")
                    nc.sync.dma_start(out=rs2, in_=rqr)
                    rbc = rtiles.tile([H, 2 * TQ], f32, tag="rbc")
                    nc.gpsimd.partition_broadcast(rbc, rs2[0:1, :])
                    for h in range(HPC):
                        zz = ztiles.tile([H, TQ], bf16, tag="zz")
                        nc.vector.tensor_mul(
                            zz, zft[0:H, h, :], rbc[:, h * TQ : (h + 1) * TQ]
                        )
                        # scatter z^T columns by destination core (2 slots of
                        # 256 per query tile)
                        for s in range(2):
                            zz_stores[b].append(
                                nc.sync.dma_start(
                                    out=a2a_in[b][
                                        2 * q + s, h * H : (h + 1) * H, :
                                    ],
                                    in_=zz[:, s * HROWS : (s + 1) * HROWS],
                                )
                            )
                # this batch's AllToAll right after its last qtile so it
                # fires as soon as the batch's z stores land
                emit_cc(b)

            # ---- output projection for this core's 256-row slice of each
            # batch (batch 0's deps resolve during batch 1's attention) ----
            for b in range(B):
                zo = [
                    singles.tile([128, HROWS], bf16, name=f"zo{b}_{g}")
                    for g in range(GC)
                ]
                for g in range(GC):
                    for s in range(2):
                        zd = nc.sync.dma_start(
                            out=zo[g][:, s * 128 : (s + 1) * 128],
                            in_=a2a_out[b][g][:, s * 128 : (s + 1) * 128],
                        )
                        add_dep_helper(
                            zd.ins, ccs[b].ins, reason="zo after a2a"
                        )
                        if b == 0:
                            # hold batch 0's zo burst until batch 1's z stores
                            # are out, so these transfers fill the second
                            # collective's barrier window instead of clogging
                            # the DMA queues under batch 1's final normalize
                            add_dep_helper(
                                zd.ins, zz_stores[1][-1].ins,
                                reason="zo0 after b1 z stores",
                            )
                for rt in range(HROWS // 128):
                    for eh in range(E // 512):
                        po = psum.tile([128, 512], f32, tag="ps")
                        for g in range(GC):
                            nc.tensor.matmul(
                                po,
                                lhsT=zo[g][:, rt * 128 : (rt + 1) * 128],
                                rhs=wo[:, g, eh * 512 : (eh + 1) * 512],
                                start=(g == 0),
                                stop=(g == GC - 1),
                            )
                        ob = otiles.tile([128, 512], bf16, tag="ob")
                        nc.vector.tensor_add(
                            ob, po, bob[:, eh * 512 : (eh + 1) * 512]
                        )
                        # split the row-tile store so the tail transfer is
                        # short and lands on four DMA engines
                        r0 = b * HROWS + rt * 128
                        for s in range(4):
                            nc.sync.dma_start(
                                out=out_d[
                                    r0 + s * 32 : r0 + (s + 1) * 32,
                                    eh * 512 : (eh + 1) * 512,
                                ],
                                in_=ob[s * 32 : (s + 1) * 32, :],
                            )

    nc.compile()
    return nc


def _prep_inputs(x, W_Q, W_K, W_V, W_O, b_Q, b_K, b_V, b_O):
    # xq [B, NQ, 128(p), EC(k), 512(j)] with element [b,q,p,k,j] =
    # x[b, q*512+j, k*128+p]
    xq = np.ascontiguousarray(
        x.reshape(B, NQ, TQ, EC, 128).transpose(0, 1, 4, 3, 2).astype(BF16)
    )
    wo = np.ascontiguousarray(
        W_O.reshape(N * H, E).reshape(GC, 128, E).astype(BF16)
    )
    bo = np.ascontiguousarray(b_O.reshape(1, E).astype(BF16))
    # causal mask for the 128-col diagonal block of a chunk: allow q >= k
    cols = np.arange(128)[None, :]
    rows = np.arange(128)[:, None]
    cmask = np.ascontiguousarray((cols >= rows).astype(BF16))

    in_maps = []
    for i in range(NCORES):
        hs = slice(HPC * i, HPC * (i + 1))
        m = {
            "xq": xq,
            "wo": wo,
            "bo": bo,
            "cmask": cmask,
            "wq": np.ascontiguousarray(
                W_Q[hs]
                .transpose(1, 0, 2)
                .reshape(EC, 128, HL)
                .transpose(1, 0, 2)
                .astype(BF16)
            ),
            "wk": np.ascontiguousarray(
                W_K[hs]
                .transpose(1, 0, 2)
                .reshape(EC, 128, HL)
                .transpose(1, 0, 2)
                .astype(BF16)
            ),
            "wv": np.ascontiguousarray(
                W_V[hs]
                .transpose(1, 0, 2)
                .reshape(EC, 128, HL)
                .transpose(1, 0, 2)
                .astype(BF16)
            ),
            "bq": np.ascontiguousarray(
                b_Q[hs].reshape(HL, 1).astype(np.float32)
            ),
            "bk": np.ascontiguousarray(
                b_K[hs].reshape(HL, 1).astype(np.float32)
            ),
            "bv": np.ascontiguousarray(
                b_V[hs].reshape(HL, 1).astype(np.float32)
            ),
        }
        in_maps.append(m)
    return in_maps


def run(inputs, trace=False):
    from concourse import bass_utils

    if "nc" not in _CACHE:
        _CACHE["nc"] = _build()
    nc = _CACHE["nc"]
    in_maps = _prep_inputs(**inputs)
    res = bass_utils.run_bass_kernel_spmd(
        nc, in_maps, core_ids=list(range(NCORES)), trace=trace
    )
    out = np.zeros((B, T, E), dtype=np.float32)
    for i, r in enumerate(res.results):
        ro = np.asarray(r["out"]).astype(np.float32)
        for b in range(B):
            out[b, i * HROWS : (i + 1) * HROWS] = ro[
                b * HROWS : (b + 1) * HROWS
            ]
    return out, res


def kernel(**inputs):
    out, _ = run(inputs, trace=False)
    return out
